# revision 47
# baseline (speedup 1.0000x reference)
"""XCA (cross-covariance) attention block on 8 TRN2 NeuronCores — v3.

Sharding: 8 cores = 4 batches x 2 image-row-halves (64 rows each).
v3: the whole q/k path runs in fp8 (e4m3) — errors wash out through the
8192-deep normalized gram contraction. fp8 DoubleRow matmuls pair the
conv's channel blocks (256-contraction), the dw taps (2 taps/matmul),
and the gram rows (2 image rows/matmul via u16-packed transposes).
v path stays bf16. Elementwise work spread over DVE/Scalar/Pool.
"""

import numpy as np
import ml_dtypes

B, C = 4, 384
HEADS, CHD = 8, 48
WP = 130              # padded row width
HR = 32               # valid rows per half
SP_H = 34 * WP        # 4420 qkv cols per half (32 rows + 2 halo)
DWLEN = 31 * WP + 128  # 4158 dw tap span per half
DWVEC = 4160           # DVE tap span (mult of 4 keeps DVE 2x/4x modes)
DWH = HR * WP         # 4160 dw storage per half
NT = 8192             # valid spatial per core
BF16 = ml_dtypes.bfloat16
F8 = ml_dtypes.float8_e4m3
SCW = 64.0            # qk conv weight scale (fp8 range use)
SCD = 2.0             # qk dw weight scale
NRM_SC = 0.0625       # Square input scale; ss scaled by 1/256 -> temp/256

# qk dw tap pairing for DoubleRow: 4 pairs (const offset delta) + 1 single
QK_PAIRS = [((0, 0), (0, 2)), ((1, 0), (1, 2)), ((2, 0), (2, 2)),
            ((0, 1), (1, 1))]
QK_SINGLE = (2, 1)
QK_SINGLE_ENG = "pe"   # 'pe' | 'pool'
# v tap split (pool: no PSUM access, no STT, and its ALU is ~15x slow —
# pool only gets the final dwv merge)
V_PE = [(0, 1), (1, 1), (2, 1), (2, 0), (2, 2), (0, 0), (1, 0), (0, 2),
        (1, 2)]
V_DVE = []
GSZ = 3               # psum tiles per weight-reuse group (LDWEIGHTS amortize)
# copy-engine rotations (s=scalar, v=vector/DVE; pool can't read PSUM)
PRE8_ROT = "v"
DW8_ROT = "v"
PRE16_ROT = "s"
OUT_ROT = "s"
VDW_ROT = "s"

_CACHE = {}


def _tiles(total, step):
    out = []
    s = 0
    while s < total:
        out.append((s, min(step, total - s)))
        s += step
    return out


def _build_body(nc, tc, tens):
    import concourse.mybir as mybir
    from concourse.ap import AP
    dt = mybir.dt
    Alu = mybir.AluOpType
    Act = mybir.ActivationFunctionType
    AX = mybir.AxisListType
    DR = mybir.MatmulPerfMode.DoubleRow

    (xd8, xd16, wqd, wvd, dtd, dtvd, dsd, wpd, idd, idud, mkd, tpd,
     outd) = tens

    cst = tc.alloc_tile_pool(name="cst", bufs=1)
    dwp = tc.alloc_tile_pool(name="dwp", bufs=1)
    wk = tc.alloc_tile_pool(name="wk", bufs=2)
    mmp = tc.alloc_tile_pool(name="mm", bufs=3, space="PSUM")
    drp = tc.alloc_tile_pool(name="dr", bufs=1, space="DRAM")
    xp8 = tc.alloc_tile_pool(name="xp8", bufs=1)
    xp16 = tc.alloc_tile_pool(name="xp16", bufs=1)
    qpp = tc.alloc_tile_pool(name="qpp", bufs=2)
    vpp = tc.alloc_tile_pool(name="vpp", bufs=2)
    accp = tc.alloc_tile_pool(name="accp", bufs=1)

    # ---- constants (ident first: feeds PE warm-up during x DMA) ----
    identb = cst.tile([128, 128], dt.bfloat16, tag="id")
    nc.sync.dma_start(out=identb[:], in_=idd.ap())
    idu = cst.tile([128, 128], dt.float8e4, tag="idu")
    nc.sync.dma_start(out=idu[:], in_=idud.ap())
    wq = cst.tile([128, 6 * 512], dt.float8e4, tag="wq")
    nc.sync.dma_start(out=wq[:], in_=wqd.ap())

    def load_x8(h):
        xs = xp8.tile([128, 3 * SP_H], dt.float8e4, tag="x8", name=f"x8h{h}")
        for (s, n) in _tiles(SP_H, 1105):
            for cb in range(3):
                nc.sync.dma_start(
                    out=xs[:, cb * SP_H + s: cb * SP_H + s + n],
                    in_=xd8.ap()[cb, :, 32 * h * WP + s: 32 * h * WP + s + n])
        return xs

    def load_x16(h):
        xs = xp16.tile([128, 3 * SP_H], dt.bfloat16, tag="x16", name=f"x16h{h}")
        for (s, n) in _tiles(SP_H, 2210):
            for cb in range(3):
                nc.sync.dma_start(
                    out=xs[:, cb * SP_H + s: cb * SP_H + s + n],
                    in_=xd16.ap()[cb, :, 32 * h * WP + s: 32 * h * WP + s + n])
        return xs

    xs8 = load_x8(0)
    dtt = cst.tile([128, 6 * 1280], dt.float8e4, tag="dtt")
    nc.sync.dma_start(out=dtt[:], in_=dtd.ap())

    wv = cst.tile([128, 3 * 384], dt.bfloat16, tag="wv")
    nc.sync.dma_start(out=wv[:], in_=wvd.ap())
    dtv = cst.tile([128, 3 * len(V_PE) * 128], dt.bfloat16, tag="dtv")
    nc.sync.dma_start(out=dtv[:], in_=dtvd.ap())
    dwsc_sb = cst.tile([128, 27], dt.float32, tag="ds")
    nc.sync.dma_start(out=dwsc_sb[:], in_=dsd.ap())
    wp_sb = cst.tile([128, 3 * 384], dt.bfloat16, tag="wp")
    nc.sync.dma_start(out=wp_sb[:], in_=wpd.ap())
    mask_sb = cst.tile([128, 3 * 384], dt.bfloat16, tag="mk")
    nc.sync.dma_start(out=mask_sb[:], in_=mkd.ap())
    tempc_sb = cst.tile([128, 4], dt.float32, tag="tc")
    nc.sync.dma_start(out=tempc_sb[:], in_=tpd.ap())
    xs16 = load_x16(0)

    cc_sb = cst.tile([128, 1160], dt.bfloat16, tag="cc")
    nc.vector.memset(cc_sb[:, 1158:1160], 0.0)
    nsum = cst.tile([128, 16], dt.float32, tag="ns")
    scr = cst.tile([128, 8], dt.float32, tag="scr")

    # PE warm-up during initial DMA (p-state ramp)
    for _ in range(12):
        wmu = mmp.tile([128, 512], dt.float32, tag="mm")
        nc.tensor.matmul(wmu[:, 0:128], lhsT=identb[:], rhs=identb[:],
                         start=True, stop=True)

    # preheat ln/exp act table set (sqrt via exp(.5 ln))
    nc.scalar.activation(scr[:, 0:1], tempc_sb[:, 0:1], Act.Ln)
    nc.scalar.activation(scr[:, 1:2], scr[:, 0:1], Act.Exp)
    nc.scalar.activation(scr[:, 2:3], scr[:, 1:2], Act.Square,
                         accum_out=scr[:, 3:4])

    # dw tiles: q g0-2, k g3-5 fp8 (per-half, reused across halves);
    # v gv0-2 bf16 (full, both halves — needed at the final GEMM)
    dwq = [dwp.tile([128, 4096], dt.float8e4, tag=f"g{i}", name=f"g{i}")
           for i in range(3)]
    dwk = [dwp.tile([128, 4096], dt.float8e4, tag=f"g{3+i}", name=f"g{3+i}")
           for i in range(3)]
    dwv = [dwp.tile([128, 2 * DWH], dt.bfloat16, tag=f"gv{i}", name=f"gv{i}")
           for i in range(3)]
    accD = accp.tile([128, DWVEC], dt.bfloat16, tag="accD")
    accT = accp.tile([128, DWVEC], dt.bfloat16, tag="accT")

    gtp = tc.alloc_tile_pool(name="gt", bufs=1, space="PSUM")
    gt_ps = [gtp.tile([128, 384], dt.float32, tag=f"gt{i}", name=f"gt{i}")
             for i in range(3)]
    tpp = tc.alloc_tile_pool(name="tp", bufs=2, space="PSUM")

    import itertools
    mmctr = itertools.count()
    rot_state = {}

    def ecopy(rot, dst, src):
        i = rot_state.get(rot, 0)
        rot_state[rot] = i + 1
        rot = VDW_ROT if rot == "VDW" else rot
        e = rot[i % len(rot)]
        if e == "s":
            nc.scalar.copy(dst, src)
        elif e == "v":
            nc.vector.tensor_copy(dst, src)
        else:
            nc.gpsimd.tensor_copy(dst, src)

    def pair_ap(base, off, delta, n):
        ps = base.ap[0][0]
        return AP(base.tensor, base.offset + off,
                  [[ps, 128], [delta, 2], [1, n]])

    pending_norm = []

    def flush_norm():
        while pending_norm:
            pending_norm.pop(0)()

    # ---------------- qk block: fp8 conv + fp8 dw (all on PE) -----------
    # unit-outer over GSZ-tile groups: one LDWEIGHTS covers GSZ matmuls
    def qkblock(h, ob, dwt, bidx, xs8_t):
        flush_norm()
        xs3 = xs8_t[:].rearrange("p (c s) -> p c s", c=3)
        pre8 = qpp.tile([128, SP_H], dt.float8e4, tag="qp")
        wdr = wq[:, ob * 512: ob * 512 + 256].rearrange(
            "p (i m) -> p i m", i=2)
        wrg = wq[:, ob * 512 + 256: ob * 512 + 512].rearrange(
            "p (i m) -> p i m", i=2)  # [zeros | W-block-2]
        tiles = _tiles(SP_H, 512)
        for g0 in range(0, len(tiles), GSZ):
            grp = tiles[g0:g0 + GSZ]
            pss = [mmp.tile([128, 512], dt.float32, tag="mm",
                            name=f"mm{next(mmctr)}") for _ in grp]
            for ps, (s, n) in zip(pss, grp):
                nc.tensor.matmul(ps[:, 0:n], lhsT=wdr,
                                 rhs=xs3[:, 0:2, s:s + n],
                                 start=True, stop=False, perf_mode=DR)
            for ps, (s, n) in zip(pss, grp):
                nc.tensor.matmul(ps[:, 0:n], lhsT=wrg,
                                 rhs=xs3[:, 1:3, s:s + n],
                                 start=False, stop=True, perf_mode=DR)
            for ps, (s, n) in zip(pss, grp):
                ecopy(PRE8_ROT, pre8[:, s:s + n], ps[:, 0:n])
        p8 = pre8[:]
        units = []
        for u, (t1, t2) in enumerate(QK_PAIRS):
            off1 = t1[0] * WP + t1[1]
            delta = (t2[0] - t1[0]) * WP + (t2[1] - t1[1])
            lt = dtt[:, ob * 1280 + u * 256: ob * 1280 + (u + 1) * 256]
            units.append((lt.rearrange("p (i m) -> p i m", i=2), off1, delta))
        off9 = QK_SINGLE[0] * WP + QK_SINGLE[1]
        lt9 = dtt[:, ob * 1280 + 1024: ob * 1280 + 1280]
        units.append((lt9.rearrange("p (i m) -> p i m", i=2), off9 - 2, 2))
        # dw output stored PACKED (128 cols/row): 3-row psum tiles,
        # strided copy drops the 2 pad cols per row
        rgrp = [(r, 3) for r in range(0, 30, 3)] + [(30, 2)]
        for g0 in range(0, len(rgrp), GSZ):
            grp = rgrp[g0:g0 + GSZ]
            pss = [mmp.tile([128, 512], dt.float32, tag="mm",
                            name=f"mm{next(mmctr)}") for _ in grp]
            for u, (lt, off1, delta) in enumerate(units):
                for ps, (r0, nr) in zip(pss, grp):
                    s = r0 * WP
                    n = (nr - 1) * WP + 128
                    nc.tensor.matmul(
                        ps[:, 0:n], lhsT=lt,
                        rhs=pair_ap(p8, off1 + s, delta, n),
                        start=(u == 0), stop=(u == len(units) - 1),
                        perf_mode=DR)
            for ps, (r0, nr) in zip(pss, grp):
                psb = ps[:]
                src = AP(psb.tensor, psb.offset,
                         [[psb.ap[0][0], 128], [WP, nr], [1, 128]])
                dst = dwt[:, r0 * 128:(r0 + nr) * 128].rearrange(
                    "p (r c) -> p r c", c=128)
                ecopy(DW8_ROT, dst, src)

        # deferred norm: ss/256 via Square(in/16); scratch reuses pre8
        def norm_fn(dwt=dwt, pre8=pre8, bidx=bidx, h=h):
            nc.scalar.activation(
                pre8[:, 0:4096], dwt[:, 0:4096], Act.Square, scale=NRM_SC,
                accum_out=nsum[:, bidx * 2 + h: bidx * 2 + h + 1])
        pending_norm.append(norm_fn)

    # ---------------- v block: bf16 conv + split dw ---------------------
    def vblock(h, vb, xs16_t):
        xs3 = xs16_t[:].rearrange("p (c s) -> p c s", c=3)
        hoff = h * DWH
        pre16 = vpp.tile([128, SP_H + 4], dt.bfloat16, tag="vp")
        nc.vector.memset(pre16[:, SP_H:SP_H + 4], 0.0)
        tiles = _tiles(SP_H, 512)
        for g0 in range(0, len(tiles), GSZ):
            grp = tiles[g0:g0 + GSZ]
            pss = [mmp.tile([128, 512], dt.float32, tag="mm",
                            name=f"mm{next(mmctr)}") for _ in grp]
            for cb in range(3):
                for ps, (s, n) in zip(pss, grp):
                    nc.tensor.matmul(
                        ps[:, 0:n],
                        lhsT=wv[:, vb * 384 + cb * 128:
                                vb * 384 + (cb + 1) * 128],
                        rhs=xs3[:, cb, s:s + n],
                        start=(cb == 0), stop=(cb == 2))
            for ps, (s, n) in zip(pss, grp):
                ecopy(PRE16_ROT, pre16[:, s:s + n], ps[:, 0:n])
        dvt = dwv[vb][:, hoff:hoff + DWVEC]
        # PE taps (dx=1) -> psum -> copy into dvt (full DWVEC span)
        tiles = _tiles(DWVEC, 512)
        for g0 in range(0, len(tiles), GSZ):
            grp = tiles[g0:g0 + GSZ]
            pss = [mmp.tile([128, 512], dt.float32, tag="mm",
                            name=f"mm{next(mmctr)}") for _ in grp]
            for t, (dy, dx) in enumerate(V_PE):
                off = dy * WP + dx
                for ps, (s, n) in zip(pss, grp):
                    nc.tensor.matmul(
                        ps[:, 0:n],
                        lhsT=dtv[:, (vb * len(V_PE) + t) * 128:
                                 (vb * len(V_PE) + t + 1) * 128],
                        rhs=pre16[:, off + s: off + s + n],
                        start=(t == 0), stop=(t == len(V_PE) - 1))
            for ps, (s, n) in zip(pss, grp):
                ecopy("VDW", dvt[:, s:s + n], ps[:, 0:n])
        if V_DVE:
            for j, (dy, dx) in enumerate(V_DVE):
                off = dy * WP + dx
                k9 = dy * 3 + dx
                dst = accD if j == 0 else accT
                nc.vector.tensor_scalar_mul(
                    dst[:], pre16[:, off:off + DWVEC],
                    dwsc_sb[:, vb * 9 + k9: vb * 9 + k9 + 1])
                if j > 0:
                    nc.vector.tensor_tensor(out=accD[:], in0=accD[:],
                                            in1=accT[:], op=Alu.add)
            nc.vector.tensor_tensor(out=dvt, in0=dvt, in1=accD[:], op=Alu.add)

    # ---- paired transposes via bf16 bitcast + DR gram ----
    # fp8 pairs ride the 16-bit transpose datapath (pure move, verified
    # exact in sim); one transpose moves 2 image rows. Gram consumes the
    # interleaved layout with stride-2 fp8 APs, 2 rows per DR matmul.
    def transposes_and_gram(h, p0, p1):
        for r2 in range(p0, p1):
            tq = tpp.tile([128, 768], dt.bfloat16, tag="tq")
            for i in range(3):
                sq = dwq[i][:].bitcast(dt.bfloat16)[:, r2 * 128:(r2 + 1) * 128]
                nc.tensor.transpose(tq[:, i * 128:(i + 1) * 128], sq,
                                    identb[:])
            for i in range(3):
                sk = dwk[i][:].bitcast(dt.bfloat16)[:, r2 * 128:(r2 + 1) * 128]
                nc.tensor.transpose(tq[:, 384 + i * 128: 384 + (i + 1) * 128],
                                    sk, identb[:])
            qt = wk.tile([128, 768], dt.bfloat16, tag="qt")
            # q: raw u16 copy (stays parity-interleaved; fine for rhs)
            nc.vector.tensor_copy(qt[:, 0:384].bitcast(dt.uint16),
                                  tq[:, 0:384].bitcast(dt.uint16))
            q8v = qt[:].bitcast(dt.float8e4)
            pstr = q8v.ap[0][0]
            t8v = tq[:].bitcast(dt.float8e4)
            tstr = t8v.ap[0][0]
            # k: deinterleave parity -> block layout (dual-fp8 LDWEIGHTS
            # requires contiguous weight sets)
            nc.scalar.copy(
                AP(q8v.tensor, q8v.offset + 768,
                   [[pstr, 128], [384, 2], [1, 384]]),
                AP(t8v.tensor, t8v.offset + 768,
                   [[tstr, 128], [1, 2], [2, 384]]))
            rhs_q = AP(q8v.tensor, q8v.offset, [[pstr, 128], [1, 2], [2, 384]])
            first = (r2 == 0)
            last = (r2 == 15)
            for i in range(3):
                lhs_k = AP(q8v.tensor, q8v.offset + 768 + i * 128,
                           [[pstr, 128], [384, 2], [1, 128]])
                nc.tensor.matmul(gt_ps[i][:], lhsT=lhs_k, rhs=rhs_q,
                                 start=first, stop=last, perf_mode=DR)

    # ------- phase A: h0 (v blocks interleave with copy-bound qk) ------
    def qk(h, j, xs_t):
        qkblock(h, j, dwq[j] if j < 3 else dwk[j - 3], j, xs_t)
    qk(0, 0, xs8)
    qk(0, 1, xs8)
    vblock(0, 0, xs16)
    qk(0, 2, xs8)
    qk(0, 3, xs8)
    vblock(0, 1, xs16)
    qk(0, 4, xs8)
    qk(0, 5, xs8)
    xs8_1 = load_x8(1)  # overlap h1 fp8 x load with h0 TG/v work
    transposes_and_gram(0, 0, 16)
    # ---- collective #1 fires right after TG; v blocks hide its latency
    flush_norm()
    for b6 in range(6):
        nc.vector.tensor_copy(cc_sb[:, 1152 + b6:1153 + b6],
                              nsum[:, b6 * 2:b6 * 2 + 1])
    for i in range(3):
        nc.vector.tensor_copy(cc_sb[:, 384 * i:384 * (i + 1)], gt_ps[i][:])
    cc1_in = drp.tile([128, 1160], dt.bfloat16, tag="cc1i")
    cc1_out = drp.tile([128, 1160], dt.bfloat16, tag="cc1o")
    nc.sync.dma_start(out=cc1_in[:], in_=cc_sb[:])
    with nc.allow_low_precision(reason="gram/norm partials all-reduced in bf16"):
        nc.gpsimd.collective_compute(
            "AllReduce", Alu.add,
            replica_groups=[[0, 1], [2, 3], [4, 5], [6, 7]],
            ins=[cc1_in.opt()], outs=[cc1_out.opt()])
    vblock(0, 2, xs16)
    xs16_1 = load_x16(1)
    # ---------------- phase B: h1 ----------------
    qk(1, 0, xs8_1)
    qk(1, 1, xs8_1)
    qk(1, 2, xs8_1)
    vblock(1, 0, xs16_1)
    qk(1, 3, xs8_1)
    qk(1, 4, xs8_1)
    vblock(1, 1, xs16_1)
    qk(1, 5, xs8_1)
    transposes_and_gram(1, 0, 16)
    flush_norm()
    for b6 in range(6):
        nc.vector.tensor_copy(cc_sb[:, 1152 + b6:1153 + b6],
                              nsum[:, b6 * 2 + 1:b6 * 2 + 2])
    for i in range(3):
        nc.vector.tensor_copy(cc_sb[:, 384 * i:384 * (i + 1)], gt_ps[i][:])
    tpp.release()
    gtp.release()

    # ---------------- collective #2 (pairwise all-reduce) --------------
    cc_in = drp.tile([128, 1160], dt.bfloat16, tag="ccin")
    cc_out = drp.tile([128, 1160], dt.bfloat16, tag="ccout")
    nc.sync.dma_start(out=cc_in[:], in_=cc_sb[:])
    with nc.allow_low_precision(reason="gram/norm partials all-reduced in bf16"):
        nc.gpsimd.collective_compute(
            "AllReduce", Alu.add,
            replica_groups=[[0, 1], [2, 3], [4, 5], [6, 7]],
            ins=[cc_in.opt()], outs=[cc_out.opt()])
    # h1 last v block fills the collective latency window
    vblock(1, 2, xs16_1)
    # fetch cc results on the sync queue (pool stays free)
    g1_sb = cst.tile([128, 1160], dt.bfloat16, tag="g1")
    nc.sync.dma_start(out=g1_sb[:], in_=cc1_out[:])
    smp = tc.alloc_tile_pool(name="smp", bufs=1)
    g_sb = cc_sb
    nc.sync.dma_start(out=g_sb[:], in_=cc_out[:])
    nc.vector.tensor_tensor(out=g_sb[:, 0:1160], in0=g_sb[:, 0:1160],
                            in1=g1_sb[:], op=Alu.add)

    # ---------------- softmax ----------------
    # rt = sqrt(ss) via exp(0.5 ln(ss)); ss carries the 1/256 scale which
    # is compensated in tempc (host divides temperature by 256)
    rt = cst.tile([128, 8], dt.float32, tag="rt")
    nc.scalar.activation(rt[:, 0:6], g_sb[:, 1152:1158], Act.Ln)
    nc.scalar.activation(rt[:, 0:6], rt[:, 0:6], Act.Exp, scale=0.5)
    nc.vector.tensor_scalar_max(rt[:, 0:6], rt[:, 0:6], 1e-12)
    inv = cst.tile([128, 8], dt.float32, tag="inv")
    nc.vector.reciprocal(inv[:, 0:6], rt[:, 0:6])

    gts_t = [cst.tile([128, 384], dt.bfloat16, tag=f"gs{i}", name=f"gsb{i}")
             for i in range(3)]
    for i in range(3):  # scale G^T rows (k channels, block i) by inv_k
        nc.vector.tensor_scalar_mul(
            gts_t[i][:], g_sb[:, 384 * i:384 * (i + 1)], inv[:, 3 + i:4 + i])
    gsp = tc.alloc_tile_pool(name="gs", bufs=1, space="PSUM")
    gs_ps = [gsp.tile([128, 384], dt.bfloat16, tag=f"gp{j}", name=f"gsp{j}")
             for j in range(3)]
    for j in range(3):
        for i in range(3):
            nc.tensor.transpose(
                gs_ps[j][:, i * 128:(i + 1) * 128],
                gts_t[i][:, j * 128:(j + 1) * 128], identb[:])
    attn_t = [cst.tile([128, 384], dt.bfloat16, tag=f"at{j}", name=f"attn{j}")
              for j in range(3)]
    sums = cst.tile([128, 4], dt.float32, tag="sm")
    zfs = [smp.tile([128, 384], dt.float32, tag=f"zf{j}", name=f"zf{j}")
           for j in range(3)]
    zs = [smp.tile([128, 384], dt.float32, tag=f"zz{j}", name=f"zz{j}")
          for j in range(3)]
    negmax = smp.tile([128, 4], dt.float32, tag="nm")
    for j in range(3):
        nc.vector.tensor_scalar_mul(zfs[j][:], gs_ps[j][:], inv[:, j:j + 1])
    for j in range(3):
        nc.vector.scalar_tensor_tensor(
            zs[j][:], zfs[j][:], tempc_sb[:, j:j + 1],
            mask_sb[:, 384 * j:384 * (j + 1)], op0=Alu.mult, op1=Alu.add)
    for j in range(3):
        nc.vector.tensor_reduce(negmax[:, j:j + 1], zs[j][:], AX.X, Alu.max,
                                negate=True)
    for j in range(3):
        nc.scalar.activation(
            attn_t[j][:], zs[j][:], Act.Exp, bias=negmax[:, j:j + 1],
            accum_out=sums[:, j:j + 1])
    gsp.release()
    invs = cst.tile([128, 4], dt.float32, tag="is")
    nc.vector.reciprocal(invs[:, 0:3], sums[:, 0:3])
    for j in range(3):
        nc.vector.tensor_scalar_mul(attn_t[j][:], attn_t[j][:], invs[:, j:j + 1])

    # keep PE warm through the softmax serial section
    warm = tc.alloc_tile_pool(name="warm", bufs=1, space="PSUM")
    wps = warm.tile([128, 384], dt.float32, tag="wm")
    nc.tensor.matmul(wps[:, 0:128], lhsT=g_sb[:, 0:128], rhs=wp_sb[:, 0:128],
                     start=True, stop=True)
    for j in range(3):
        nc.tensor.matmul(wps[:, 0:128], lhsT=gts_t[j][:, 0:128],
                         rhs=wp_sb[:, 0:128], start=True, stop=True)
        nc.tensor.matmul(wps[:, 0:128], lhsT=attn_t[j][:, 0:128],
                         rhs=wp_sb[:, 0:128], start=True, stop=True)
    warm.release()

    # ---------------- M = A^T Wp^T  [d, o] ----------------
    mp = tc.alloc_tile_pool(name="mp", bufs=1, space="PSUM")
    m_ps = [mp.tile([128, 384], dt.float32, tag=f"m{i}", name=f"mps{i}")
            for i in range(3)]
    for db in range(3):
        cbs = [cb for cb in range(3) if abs(cb - db) <= 1]
        for idx, cb in enumerate(cbs):
            nc.tensor.matmul(
                m_ps[db][:], lhsT=attn_t[cb][:, db * 128:(db + 1) * 128],
                rhs=wp_sb[:, cb * 384:(cb + 1) * 384],
                start=(idx == 0), stop=(idx == len(cbs) - 1))
    m_sb = cst.tile([128, 3 * 384], dt.bfloat16, tag="msb")
    for db in range(3):
        nc.scalar.copy(m_sb[:, db * 384:(db + 1) * 384], m_ps[db][:])
    mp.release()

    # ---------------- out = M^T @ V (bf16 out) + store ----------------
    otp = tc.alloc_tile_pool(name="otp", bufs=2)
    for ob in range(3):
        for t in range(16):
            ps = mmp.tile([128, 512], dt.float32, tag="mm")
            hh, r4 = t // 8, (t % 8) * 4
            for db in range(3):
                vv = dwv[db][:, hh * DWH:(hh + 1) * DWH].rearrange(
                    "p (r c) -> p r c", c=WP)
                nc.tensor.matmul(
                    ps[:],
                    lhsT=m_sb[:, db * 384 + ob * 128: db * 384 + ob * 128 + 128],
                    rhs=vv[:, r4:r4 + 4, 0:128],
                    start=(db == 0), stop=(db == 2))
            ot = otp.tile([128, 512], dt.bfloat16, tag="ot")
            ecopy(OUT_ROT, ot[:], ps[:])
            nc.sync.dma_start(
                out=outd.ap()[ob, :, 512 * t:512 * (t + 1)], in_=ot[:])

    for p in (otp, smp, drp, mmp, accp, vpp, qpp, xp16, xp8, wk, dwp, cst):
        p.release()


def build_nc():
    if "nc" in _CACHE:
        return _CACHE["nc"]
    from concourse import bacc, tile
    import concourse.mybir as mybir
    dt = mybir.dt
    nc = bacc.Bacc("TRN2", target_bir_lowering=False, debug=False, num_devices=8)
    xd8 = nc.dram_tensor("x8", [3, 128, 66 * WP], dt.float8e4,
                         kind="ExternalInput")
    xd16 = nc.dram_tensor("x16", [3, 128, 66 * WP], dt.bfloat16,
                          kind="ExternalInput")
    wqd = nc.dram_tensor("wq8", [128, 6 * 512], dt.float8e4,
                         kind="ExternalInput")
    wvd = nc.dram_tensor("wv", [128, 3 * 384], dt.bfloat16,
                         kind="ExternalInput")
    dtd = nc.dram_tensor("dt8", [128, 6 * 1280], dt.float8e4,
                         kind="ExternalInput")
    dtvd = nc.dram_tensor("dtv", [128, 3 * len(V_PE) * 128], dt.bfloat16,
                          kind="ExternalInput")
    dsd = nc.dram_tensor("dwsc", [128, 27], dt.float32,
                         kind="ExternalInput")
    wpd = nc.dram_tensor("wp", [128, 3 * 384], dt.bfloat16,
                         kind="ExternalInput")
    idd = nc.dram_tensor("identb", [128, 128], dt.bfloat16,
                         kind="ExternalInput")
    idud = nc.dram_tensor("idu", [128, 128], dt.float8e4,
                          kind="ExternalInput")
    mkd = nc.dram_tensor("maskt", [128, 3 * 384], dt.bfloat16,
                         kind="ExternalInput")
    tpd = nc.dram_tensor("tempc", [128, 4], dt.float32, kind="ExternalInput")
    outd = nc.dram_tensor("out", [3, 128, NT], dt.bfloat16,
                          kind="ExternalOutput")
    with tile.TileContext(nc) as tc:
        _build_body(nc, tc, (xd8, xd16, wqd, wvd, dtd, dtvd, dsd, wpd, idd,
                             idud, mkd, tpd, outd))
    nc.compile()
    _CACHE["nc"] = nc
    return nc


def make_in_maps(x, qkv_w, dw_w, proj_w, temperature):
    x = np.asarray(x, np.float32)
    qkv_w = np.asarray(qkv_w, np.float32)
    dw_w = np.asarray(dw_w, np.float32)
    proj_w = np.asarray(proj_w, np.float32)
    temperature = np.asarray(temperature, np.float32).reshape(-1)

    xp = np.zeros((B, C, 130, 130), np.float32)
    xp[:, :, 1:129, 1:129] = x

    rng = np.arange(128)
    # qk conv weights, fp8 DoubleRow; unit2 = [zeros | W-block-2]
    wq8 = np.zeros((128, 6 * 512), np.float32)
    for ob in range(6):
        blk = qkv_w[ob * 128:(ob + 1) * 128] * SCW  # [128m, 384c]
        for i in range(2):
            wq8[:, ob * 512 + i * 128: ob * 512 + (i + 1) * 128] = \
                blk[:, i * 128:(i + 1) * 128].T
        wq8[:, ob * 512 + 384: ob * 512 + 512] = blk[:, 256:384].T
    # v conv weights bf16 (classic layout)
    wv16 = np.zeros((128, 3 * 384), np.float32)
    for vb in range(3):
        for cb in range(3):
            blk = qkv_w[768 + vb * 128: 768 + (vb + 1) * 128,
                        cb * 128:(cb + 1) * 128]
            wv16[:, vb * 384 + cb * 128: vb * 384 + (cb + 1) * 128] = blk.T
    # qk dw tap diagonals, fp8 DoubleRow; unit4 = [zeros | single tap]
    dt8 = np.zeros((128, 6 * 1280), np.float32)
    for ob in range(6):
        for u, (t1, t2) in enumerate(QK_PAIRS):
            c0 = ob * 1280 + u * 256
            dt8[rng, c0 + rng] = dw_w[ob * 128 + rng, 0, t1[0], t1[1]] * SCD
            dt8[rng, c0 + 128 + rng] = dw_w[ob * 128 + rng, 0, t2[0], t2[1]] * SCD
        c0 = ob * 1280 + 1024 + 128
        dt8[rng, c0 + rng] = dw_w[ob * 128 + rng, 0, QK_SINGLE[0],
                                  QK_SINGLE[1]] * SCD
    # v dw PE tap diagonals bf16
    dtv16 = np.zeros((128, 3 * len(V_PE) * 128), np.float32)
    for vb in range(3):
        for t, (dy, dx) in enumerate(V_PE):
            c0 = (vb * len(V_PE) + t) * 128
            dtv16[rng, c0 + rng] = dw_w[768 + vb * 128 + rng, 0, dy, dx]
    # v dw per-channel scales (DVE/pool taps)
    dwsc = np.zeros((128, 27), np.float32)
    for vb in range(3):
        for k9 in range(9):
            dwsc[:, vb * 9 + k9] = dw_w[768 + vb * 128:768 + (vb + 1) * 128,
                                        0, k9 // 3, k9 % 3]
    wpm = np.zeros((128, 3 * 384), np.float32)
    for cb in range(3):
        wpm[:, cb * 384:(cb + 1) * 384] = proj_w[:, cb * 128:(cb + 1) * 128].T
    ident = np.eye(128, dtype=np.float32)
    idu8 = np.eye(128, dtype=np.float32).astype(F8)
    mask = np.full((128, 3 * 384), -1e30, np.float32)
    for j in range(3):
        for p in range(128):
            hgrp = (128 * j + p) // CHD
            mask[p, 384 * j + CHD * hgrp: 384 * j + CHD * (hgrp + 1)] = 0.0
    tempc = np.zeros((128, 4), np.float32)
    for j in range(3):
        for p in range(128):
            tempc[p, j] = temperature[(128 * j + p) // CHD] / 256.0

    shared = {
        "wq8": wq8.astype(F8), "wv": wv16.astype(BF16),
        "dt8": dt8.astype(F8), "dtv": dtv16.astype(BF16),
        "dwsc": dwsc,
        "wp": wpm.astype(BF16), "identb": ident.astype(BF16), "idu": idu8,
        "maskt": mask.astype(BF16), "tempc": tempc,
    }
    in_maps = []
    for core in range(8):
        b, s = core // 2, core % 2
        xs = xp[b, :, 64 * s: 64 * s + 66, :]
        xs = np.ascontiguousarray(xs.reshape(3, 128, 66 * WP))
        m = dict(shared)
        m["x8"] = xs.astype(F8)
        m["x16"] = xs.astype(BF16)
        in_maps.append(m)
    return in_maps


def assemble(results):
    full = np.zeros((B, C, 128, 128), np.float32)
    for core in range(8):
        b, s = core // 2, core % 2
        o = np.asarray(results[core]["out"], np.float32).reshape(C, 64, 128)
        full[b, :, 64 * s: 64 * s + 64, :] = o
    return full


def kernel(x, qkv_w, dw_w, proj_w, temperature):
    from concourse.bass_utils import run_bass_kernel_spmd
    nc = build_nc()
    in_maps = make_in_maps(x, qkv_w, dw_w, proj_w, temperature)
    res = run_bass_kernel_spmd(nc, in_maps, core_ids=list(range(8)))
    return assemble(res.results)


# revision 48
# speedup vs baseline: 1.1199x; 1.1199x over previous
"""XCA (cross-covariance) attention block on 8 TRN2 NeuronCores — v3.

Sharding: 8 cores = 4 batches x 2 image-row-halves (64 rows each).
v3: the whole q/k path runs in fp8 (e4m3) — errors wash out through the
8192-deep normalized gram contraction. fp8 DoubleRow matmuls pair the
conv's channel blocks (256-contraction), the dw taps (2 taps/matmul),
and the gram rows (2 image rows/matmul via u16-packed transposes).
v path stays bf16. Elementwise work spread over DVE/Scalar/Pool.
"""

import numpy as np
import ml_dtypes

B, C = 4, 384
HEADS, CHD = 8, 48
WP = 130              # padded row width
HR = 32               # valid rows per half
SP_H = 34 * WP        # 4420 qkv cols per half (32 rows + 2 halo)
DWLEN = 31 * WP + 128  # 4158 dw tap span per half
DWVEC = 4160           # DVE tap span (mult of 4 keeps DVE 2x/4x modes)
DWH = HR * WP         # 4160 dw storage per half
NT = 8192             # valid spatial per core
BF16 = ml_dtypes.bfloat16
F8 = ml_dtypes.float8_e4m3
SCW = 64.0            # qk conv weight scale (fp8 range use)
SCD = 2.0             # qk dw weight scale
NRM_SC = 0.0625       # Square input scale; ss scaled by 1/256 -> temp/256

# qk dw tap pairing for DoubleRow: 4 pairs (const offset delta) + 1 single
QK_PAIRS = [((0, 0), (0, 2)), ((1, 0), (1, 2)), ((2, 0), (2, 2)),
            ((0, 1), (1, 1))]
QK_SINGLE = (2, 1)
QK_SINGLE_ENG = "pe"   # 'pe' | 'pool'
# v tap split (pool: no PSUM access, no STT, and its ALU is ~15x slow —
# pool only gets the final dwv merge)
V_PE = [(0, 1), (1, 1), (2, 1), (2, 0), (2, 2), (0, 0), (1, 0)]
V_DVE = [(0, 2), (1, 2)]
GSZ = 3               # psum tiles per weight-reuse group (LDWEIGHTS amortize)
# copy-engine rotations (s=scalar, v=vector/DVE; pool can't read PSUM)
PRE8_ROT = "v"
DW8_ROT = "v"
PRE16_ROT = "vs"
OUT_ROT = "s"
VDW_ROT = "v"

_CACHE = {}


def _tiles(total, step):
    out = []
    s = 0
    while s < total:
        out.append((s, min(step, total - s)))
        s += step
    return out


def _build_body(nc, tc, tens):
    import concourse.mybir as mybir
    from concourse.ap import AP
    dt = mybir.dt
    Alu = mybir.AluOpType
    Act = mybir.ActivationFunctionType
    AX = mybir.AxisListType
    DR = mybir.MatmulPerfMode.DoubleRow

    (xd8, xd16, wqd, wvd, dtd, dtvd, dsd, wpd, idd, idud, mkd, tpd,
     outd) = tens

    cst = tc.alloc_tile_pool(name="cst", bufs=1)
    dwp = tc.alloc_tile_pool(name="dwp", bufs=1)
    wk = tc.alloc_tile_pool(name="wk", bufs=2)
    mmp = tc.alloc_tile_pool(name="mm", bufs=3, space="PSUM")
    drp = tc.alloc_tile_pool(name="dr", bufs=1, space="DRAM")
    xp8 = tc.alloc_tile_pool(name="xp8", bufs=1)
    xp16 = tc.alloc_tile_pool(name="xp16", bufs=1)
    qpp = tc.alloc_tile_pool(name="qpp", bufs=2)
    vpp = tc.alloc_tile_pool(name="vpp", bufs=2)
    accp = tc.alloc_tile_pool(name="accp", bufs=1)

    # ---- constants (ident first: feeds PE warm-up during x DMA) ----
    identb = cst.tile([128, 128], dt.bfloat16, tag="id")
    nc.sync.dma_start(out=identb[:], in_=idd.ap())
    idu = cst.tile([128, 128], dt.float8e4, tag="idu")
    nc.sync.dma_start(out=idu[:], in_=idud.ap())
    wq = cst.tile([128, 6 * 512], dt.float8e4, tag="wq")
    nc.sync.dma_start(out=wq[:], in_=wqd.ap())

    def load_x8(h):
        xs = xp8.tile([128, 3 * SP_H], dt.float8e4, tag="x8", name=f"x8h{h}")
        for (s, n) in _tiles(SP_H, 1105):
            for cb in range(3):
                nc.sync.dma_start(
                    out=xs[:, cb * SP_H + s: cb * SP_H + s + n],
                    in_=xd8.ap()[cb, :, 32 * h * WP + s: 32 * h * WP + s + n])
        return xs

    def load_x16(h):
        xs = xp16.tile([128, 3 * SP_H], dt.bfloat16, tag="x16", name=f"x16h{h}")
        for (s, n) in _tiles(SP_H, 2210):
            for cb in range(3):
                nc.sync.dma_start(
                    out=xs[:, cb * SP_H + s: cb * SP_H + s + n],
                    in_=xd16.ap()[cb, :, 32 * h * WP + s: 32 * h * WP + s + n])
        return xs

    xs8 = load_x8(0)
    dtt = cst.tile([128, 6 * 1280], dt.float8e4, tag="dtt")
    nc.sync.dma_start(out=dtt[:], in_=dtd.ap())

    wv = cst.tile([128, 3 * 384], dt.bfloat16, tag="wv")
    nc.sync.dma_start(out=wv[:], in_=wvd.ap())
    dtv = cst.tile([128, 3 * len(V_PE) * 128], dt.bfloat16, tag="dtv")
    nc.sync.dma_start(out=dtv[:], in_=dtvd.ap())
    dwsc_sb = cst.tile([128, 27], dt.float32, tag="ds")
    nc.sync.dma_start(out=dwsc_sb[:], in_=dsd.ap())
    wp_sb = cst.tile([128, 3 * 384], dt.bfloat16, tag="wp")
    nc.sync.dma_start(out=wp_sb[:], in_=wpd.ap())
    mask_sb = cst.tile([128, 3 * 384], dt.bfloat16, tag="mk")
    nc.sync.dma_start(out=mask_sb[:], in_=mkd.ap())
    tempc_sb = cst.tile([128, 4], dt.float32, tag="tc")
    nc.sync.dma_start(out=tempc_sb[:], in_=tpd.ap())
    xs16 = load_x16(0)

    cc_sb = cst.tile([128, 1160], dt.bfloat16, tag="cc")
    nc.vector.memset(cc_sb[:, 1158:1160], 0.0)
    nsum = cst.tile([128, 16], dt.float32, tag="ns")
    scr = cst.tile([128, 8], dt.float32, tag="scr")

    # PE warm-up during initial DMA (p-state ramp)
    for _ in range(12):
        wmu = mmp.tile([128, 512], dt.float32, tag="mm")
        nc.tensor.matmul(wmu[:, 0:128], lhsT=identb[:], rhs=identb[:],
                         start=True, stop=True)

    # preheat ln/exp act table set (sqrt via exp(.5 ln))
    nc.scalar.activation(scr[:, 0:1], tempc_sb[:, 0:1], Act.Ln)
    nc.scalar.activation(scr[:, 1:2], scr[:, 0:1], Act.Exp)
    nc.scalar.activation(scr[:, 2:3], scr[:, 1:2], Act.Square,
                         accum_out=scr[:, 3:4])

    # dw tiles: q g0-2, k g3-5 fp8 (per-half, reused across halves);
    # v gv0-2 bf16 (full, both halves — needed at the final GEMM)
    dwq = [dwp.tile([128, 4096], dt.float8e4, tag=f"g{i}", name=f"g{i}")
           for i in range(3)]
    dwk = [dwp.tile([128, 4096], dt.float8e4, tag=f"g{3+i}", name=f"g{3+i}")
           for i in range(3)]
    dwv = [dwp.tile([128, 2 * DWH], dt.bfloat16, tag=f"gv{i}", name=f"gv{i}")
           for i in range(3)]
    accD = accp.tile([128, DWVEC], dt.bfloat16, tag="accD")
    accT = accp.tile([128, DWVEC], dt.bfloat16, tag="accT")

    gtp = tc.alloc_tile_pool(name="gt", bufs=1, space="PSUM")
    gt_ps = [gtp.tile([128, 384], dt.float32, tag=f"gt{i}", name=f"gt{i}")
             for i in range(3)]
    tpp = tc.alloc_tile_pool(name="tp", bufs=2, space="PSUM")

    import itertools
    mmctr = itertools.count()
    rot_state = {}

    def ecopy(rot, dst, src):
        i = rot_state.get(rot, 0)
        rot_state[rot] = i + 1
        rot = VDW_ROT if rot == "VDW" else rot
        e = rot[i % len(rot)]
        if e == "s":
            nc.scalar.copy(dst, src)
        elif e == "v":
            nc.vector.tensor_copy(dst, src)
        else:
            nc.gpsimd.tensor_copy(dst, src)

    def pair_ap(base, off, delta, n):
        ps = base.ap[0][0]
        return AP(base.tensor, base.offset + off,
                  [[ps, 128], [delta, 2], [1, n]])

    pending_norm = []

    def flush_norm():
        while pending_norm:
            pending_norm.pop(0)()

    # ---------------- qk block: fp8 conv + fp8 dw (all on PE) -----------
    # unit-outer over GSZ-tile groups: one LDWEIGHTS covers GSZ matmuls
    def qkblock(h, ob, dwt, bidx, xs8_t):
        flush_norm()
        xs3 = xs8_t[:].rearrange("p (c s) -> p c s", c=3)
        pre8 = qpp.tile([128, SP_H], dt.float8e4, tag="qp")
        wdr = wq[:, ob * 512: ob * 512 + 256].rearrange(
            "p (i m) -> p i m", i=2)
        wrg = wq[:, ob * 512 + 256: ob * 512 + 512].rearrange(
            "p (i m) -> p i m", i=2)  # [zeros | W-block-2]
        tiles = _tiles(SP_H, 512)
        for g0 in range(0, len(tiles), GSZ):
            grp = tiles[g0:g0 + GSZ]
            pss = [mmp.tile([128, 512], dt.float32, tag="mm",
                            name=f"mm{next(mmctr)}") for _ in grp]
            for ps, (s, n) in zip(pss, grp):
                nc.tensor.matmul(ps[:, 0:n], lhsT=wdr,
                                 rhs=xs3[:, 0:2, s:s + n],
                                 start=True, stop=False, perf_mode=DR)
            for ps, (s, n) in zip(pss, grp):
                nc.tensor.matmul(ps[:, 0:n], lhsT=wrg,
                                 rhs=xs3[:, 1:3, s:s + n],
                                 start=False, stop=True, perf_mode=DR)
            for ps, (s, n) in zip(pss, grp):
                ecopy(PRE8_ROT, pre8[:, s:s + n], ps[:, 0:n])
        p8 = pre8[:]
        units = []
        for u, (t1, t2) in enumerate(QK_PAIRS):
            off1 = t1[0] * WP + t1[1]
            delta = (t2[0] - t1[0]) * WP + (t2[1] - t1[1])
            lt = dtt[:, ob * 1280 + u * 256: ob * 1280 + (u + 1) * 256]
            units.append((lt.rearrange("p (i m) -> p i m", i=2), off1, delta))
        off9 = QK_SINGLE[0] * WP + QK_SINGLE[1]
        lt9 = dtt[:, ob * 1280 + 1024: ob * 1280 + 1280]
        units.append((lt9.rearrange("p (i m) -> p i m", i=2), off9 - 2, 2))
        # dw output stored PACKED (128 cols/row): 3-row psum tiles,
        # strided copy drops the 2 pad cols per row
        rgrp = [(r, 3) for r in range(0, 30, 3)] + [(30, 2)]
        for g0 in range(0, len(rgrp), GSZ):
            grp = rgrp[g0:g0 + GSZ]
            pss = [mmp.tile([128, 512], dt.float32, tag="mm",
                            name=f"mm{next(mmctr)}") for _ in grp]
            for u, (lt, off1, delta) in enumerate(units):
                for ps, (r0, nr) in zip(pss, grp):
                    s = r0 * WP
                    n = (nr - 1) * WP + 128
                    nc.tensor.matmul(
                        ps[:, 0:n], lhsT=lt,
                        rhs=pair_ap(p8, off1 + s, delta, n),
                        start=(u == 0), stop=(u == len(units) - 1),
                        perf_mode=DR)
            for ps, (r0, nr) in zip(pss, grp):
                psb = ps[:]
                src = AP(psb.tensor, psb.offset,
                         [[psb.ap[0][0], 128], [WP, nr], [1, 128]])
                dst = dwt[:, r0 * 128:(r0 + nr) * 128].rearrange(
                    "p (r c) -> p r c", c=128)
                ecopy(DW8_ROT, dst, src)

        # deferred norm: ss/256 via Square(in/16); scratch reuses pre8
        def norm_fn(dwt=dwt, pre8=pre8, bidx=bidx, h=h):
            nc.scalar.activation(
                pre8[:, 0:4096], dwt[:, 0:4096], Act.Square, scale=NRM_SC,
                accum_out=nsum[:, bidx * 2 + h: bidx * 2 + h + 1])
        pending_norm.append(norm_fn)

    # ---------------- v block: bf16 conv + split dw ---------------------
    def vblock(h, vb, xs16_t):
        xs3 = xs16_t[:].rearrange("p (c s) -> p c s", c=3)
        hoff = h * DWH
        pre16 = vpp.tile([128, SP_H + 4], dt.bfloat16, tag="vp")
        nc.vector.memset(pre16[:, SP_H:SP_H + 4], 0.0)
        tiles = _tiles(SP_H, 512)
        for g0 in range(0, len(tiles), GSZ):
            grp = tiles[g0:g0 + GSZ]
            pss = [mmp.tile([128, 512], dt.float32, tag="mm",
                            name=f"mm{next(mmctr)}") for _ in grp]
            for cb in range(3):
                for ps, (s, n) in zip(pss, grp):
                    nc.tensor.matmul(
                        ps[:, 0:n],
                        lhsT=wv[:, vb * 384 + cb * 128:
                                vb * 384 + (cb + 1) * 128],
                        rhs=xs3[:, cb, s:s + n],
                        start=(cb == 0), stop=(cb == 2))
            for ps, (s, n) in zip(pss, grp):
                ecopy(PRE16_ROT, pre16[:, s:s + n], ps[:, 0:n])
        dvt = dwv[vb][:, hoff:hoff + DWVEC]
        # PE taps (dx=1) -> psum -> copy into dvt (full DWVEC span)
        tiles = _tiles(DWVEC, 512)
        for g0 in range(0, len(tiles), GSZ):
            grp = tiles[g0:g0 + GSZ]
            pss = [mmp.tile([128, 512], dt.float32, tag="mm",
                            name=f"mm{next(mmctr)}") for _ in grp]
            for t, (dy, dx) in enumerate(V_PE):
                off = dy * WP + dx
                for ps, (s, n) in zip(pss, grp):
                    nc.tensor.matmul(
                        ps[:, 0:n],
                        lhsT=dtv[:, (vb * len(V_PE) + t) * 128:
                                 (vb * len(V_PE) + t + 1) * 128],
                        rhs=pre16[:, off + s: off + s + n],
                        start=(t == 0), stop=(t == len(V_PE) - 1))
            for ps, (s, n) in zip(pss, grp):
                ecopy("VDW", dvt[:, s:s + n], ps[:, 0:n])
        if V_DVE:
            for j, (dy, dx) in enumerate(V_DVE):
                off = dy * WP + dx
                k9 = dy * 3 + dx
                dst = accD if j == 0 else accT
                nc.vector.tensor_scalar_mul(
                    dst[:], pre16[:, off:off + DWVEC],
                    dwsc_sb[:, vb * 9 + k9: vb * 9 + k9 + 1])
                if j > 0:
                    nc.vector.tensor_tensor(out=accD[:], in0=accD[:],
                                            in1=accT[:], op=Alu.add)
            nc.vector.tensor_tensor(out=dvt, in0=dvt, in1=accD[:], op=Alu.add)

    # ---- paired transposes via bf16 bitcast + DR gram ----
    # fp8 pairs ride the 16-bit transpose datapath (pure move, verified
    # exact in sim); one transpose moves 2 image rows. Gram consumes the
    # interleaved layout with stride-2 fp8 APs, 2 rows per DR matmul.
    def transposes_and_gram(h, p0, p1):
        for r2 in range(p0, p1):
            tq = tpp.tile([128, 768], dt.bfloat16, tag="tq")
            for i in range(3):
                sq = dwq[i][:].bitcast(dt.bfloat16)[:, r2 * 128:(r2 + 1) * 128]
                nc.tensor.transpose(tq[:, i * 128:(i + 1) * 128], sq,
                                    identb[:])
            for i in range(3):
                sk = dwk[i][:].bitcast(dt.bfloat16)[:, r2 * 128:(r2 + 1) * 128]
                nc.tensor.transpose(tq[:, 384 + i * 128: 384 + (i + 1) * 128],
                                    sk, identb[:])
            qt = wk.tile([128, 768], dt.bfloat16, tag="qt")
            # q: raw u16 copy (stays parity-interleaved; fine for rhs)
            nc.vector.tensor_copy(qt[:, 0:384].bitcast(dt.uint16),
                                  tq[:, 0:384].bitcast(dt.uint16))
            q8v = qt[:].bitcast(dt.float8e4)
            pstr = q8v.ap[0][0]
            t8v = tq[:].bitcast(dt.float8e4)
            tstr = t8v.ap[0][0]
            # k: deinterleave parity -> block layout (dual-fp8 LDWEIGHTS
            # requires contiguous weight sets)
            nc.scalar.copy(
                AP(q8v.tensor, q8v.offset + 768,
                   [[pstr, 128], [384, 2], [1, 384]]),
                AP(t8v.tensor, t8v.offset + 768,
                   [[tstr, 128], [1, 2], [2, 384]]))
            rhs_q = AP(q8v.tensor, q8v.offset, [[pstr, 128], [1, 2], [2, 384]])
            first = (r2 == 0)
            last = (r2 == 15)
            for i in range(3):
                lhs_k = AP(q8v.tensor, q8v.offset + 768 + i * 128,
                           [[pstr, 128], [384, 2], [1, 128]])
                nc.tensor.matmul(gt_ps[i][:], lhsT=lhs_k, rhs=rhs_q,
                                 start=first, stop=last, perf_mode=DR)

    # ---------------- phase A: h0 ----------------
    def qk(h, j, xs_t):
        qkblock(h, j, dwq[j] if j < 3 else dwk[j - 3], j, xs_t)
    for j in range(6):
        qk(0, j, xs8)
    xs8_1 = load_x8(1)  # overlap h1 fp8 x load with h0 TG/v work
    transposes_and_gram(0, 0, 16)
    vblock(0, 0, xs16)
    # ---- collective #1 fires right after TG; v blocks hide its latency
    flush_norm()
    for b6 in range(6):
        nc.vector.tensor_copy(cc_sb[:, 1152 + b6:1153 + b6],
                              nsum[:, b6 * 2:b6 * 2 + 1])
    for i in range(3):
        nc.vector.tensor_copy(cc_sb[:, 384 * i:384 * (i + 1)], gt_ps[i][:])
    cc1_in = drp.tile([128, 1160], dt.bfloat16, tag="cc1i")
    cc1_out = drp.tile([128, 1160], dt.bfloat16, tag="cc1o")
    nc.sync.dma_start(out=cc1_in[:], in_=cc_sb[:])
    with nc.allow_low_precision(reason="gram/norm partials all-reduced in bf16"):
        nc.gpsimd.collective_compute(
            "AllReduce", Alu.add,
            replica_groups=[[0, 1], [2, 3], [4, 5], [6, 7]],
            ins=[cc1_in.opt()], outs=[cc1_out.opt()])
    vblock(0, 1, xs16)
    vblock(0, 2, xs16)
    xs16_1 = load_x16(1)
    # ---------------- phase B: h1 ----------------
    for j in range(6):
        qk(1, j, xs8_1)
    transposes_and_gram(1, 0, 16)
    vblock(1, 0, xs16_1)
    flush_norm()
    for b6 in range(6):
        nc.vector.tensor_copy(cc_sb[:, 1152 + b6:1153 + b6],
                              nsum[:, b6 * 2 + 1:b6 * 2 + 2])
    for i in range(3):
        nc.vector.tensor_copy(cc_sb[:, 384 * i:384 * (i + 1)], gt_ps[i][:])
    tpp.release()
    gtp.release()

    # ---------------- collective #2 (pairwise all-reduce) --------------
    cc_in = drp.tile([128, 1160], dt.bfloat16, tag="ccin")
    cc_out = drp.tile([128, 1160], dt.bfloat16, tag="ccout")
    nc.sync.dma_start(out=cc_in[:], in_=cc_sb[:])
    with nc.allow_low_precision(reason="gram/norm partials all-reduced in bf16"):
        nc.gpsimd.collective_compute(
            "AllReduce", Alu.add,
            replica_groups=[[0, 1], [2, 3], [4, 5], [6, 7]],
            ins=[cc_in.opt()], outs=[cc_out.opt()])
    # h1 v blocks fill the collective latency window
    vblock(1, 1, xs16_1)
    vblock(1, 2, xs16_1)
    # fetch cc results on the sync queue (pool stays free)
    g1_sb = cst.tile([128, 1160], dt.bfloat16, tag="g1")
    nc.sync.dma_start(out=g1_sb[:], in_=cc1_out[:])
    smp = tc.alloc_tile_pool(name="smp", bufs=1)
    g_sb = cc_sb
    nc.sync.dma_start(out=g_sb[:], in_=cc_out[:])
    nc.vector.tensor_tensor(out=g_sb[:, 0:1160], in0=g_sb[:, 0:1160],
                            in1=g1_sb[:], op=Alu.add)

    # ---------------- softmax ----------------
    # rt = sqrt(ss) via exp(0.5 ln(ss)); ss carries the 1/256 scale which
    # is compensated in tempc (host divides temperature by 256)
    rt = cst.tile([128, 8], dt.float32, tag="rt")
    nc.scalar.activation(rt[:, 0:6], g_sb[:, 1152:1158], Act.Ln)
    nc.scalar.activation(rt[:, 0:6], rt[:, 0:6], Act.Exp, scale=0.5)
    nc.vector.tensor_scalar_max(rt[:, 0:6], rt[:, 0:6], 1e-12)
    inv = cst.tile([128, 8], dt.float32, tag="inv")
    nc.vector.reciprocal(inv[:, 0:6], rt[:, 0:6])

    gts_t = [cst.tile([128, 384], dt.bfloat16, tag=f"gs{i}", name=f"gsb{i}")
             for i in range(3)]
    for i in range(3):  # scale G^T rows (k channels, block i) by inv_k
        nc.vector.tensor_scalar_mul(
            gts_t[i][:], g_sb[:, 384 * i:384 * (i + 1)], inv[:, 3 + i:4 + i])
    gsp = tc.alloc_tile_pool(name="gs", bufs=1, space="PSUM")
    gs_ps = [gsp.tile([128, 384], dt.bfloat16, tag=f"gp{j}", name=f"gsp{j}")
             for j in range(3)]
    for j in range(3):
        for i in range(3):
            nc.tensor.transpose(
                gs_ps[j][:, i * 128:(i + 1) * 128],
                gts_t[i][:, j * 128:(j + 1) * 128], identb[:])
    attn_t = [cst.tile([128, 384], dt.bfloat16, tag=f"at{j}", name=f"attn{j}")
              for j in range(3)]
    sums = cst.tile([128, 4], dt.float32, tag="sm")
    zfs = [smp.tile([128, 384], dt.float32, tag=f"zf{j}", name=f"zf{j}")
           for j in range(3)]
    zs = [smp.tile([128, 384], dt.float32, tag=f"zz{j}", name=f"zz{j}")
          for j in range(3)]
    negmax = smp.tile([128, 4], dt.float32, tag="nm")
    for j in range(3):
        nc.vector.tensor_scalar_mul(zfs[j][:], gs_ps[j][:], inv[:, j:j + 1])
    for j in range(3):
        nc.vector.scalar_tensor_tensor(
            zs[j][:], zfs[j][:], tempc_sb[:, j:j + 1],
            mask_sb[:, 384 * j:384 * (j + 1)], op0=Alu.mult, op1=Alu.add)
    for j in range(3):
        nc.vector.tensor_reduce(negmax[:, j:j + 1], zs[j][:], AX.X, Alu.max,
                                negate=True)
    for j in range(3):
        nc.scalar.activation(
            attn_t[j][:], zs[j][:], Act.Exp, bias=negmax[:, j:j + 1],
            accum_out=sums[:, j:j + 1])
    gsp.release()
    invs = cst.tile([128, 4], dt.float32, tag="is")
    nc.vector.reciprocal(invs[:, 0:3], sums[:, 0:3])
    for j in range(3):
        nc.vector.tensor_scalar_mul(attn_t[j][:], attn_t[j][:], invs[:, j:j + 1])

    # keep PE warm through the softmax serial section
    warm = tc.alloc_tile_pool(name="warm", bufs=1, space="PSUM")
    wps = warm.tile([128, 384], dt.float32, tag="wm")
    nc.tensor.matmul(wps[:, 0:128], lhsT=g_sb[:, 0:128], rhs=wp_sb[:, 0:128],
                     start=True, stop=True)
    for j in range(3):
        nc.tensor.matmul(wps[:, 0:128], lhsT=gts_t[j][:, 0:128],
                         rhs=wp_sb[:, 0:128], start=True, stop=True)
        nc.tensor.matmul(wps[:, 0:128], lhsT=attn_t[j][:, 0:128],
                         rhs=wp_sb[:, 0:128], start=True, stop=True)
    warm.release()

    # ---------------- M = A^T Wp^T  [d, o] ----------------
    mp = tc.alloc_tile_pool(name="mp", bufs=1, space="PSUM")
    m_ps = [mp.tile([128, 384], dt.float32, tag=f"m{i}", name=f"mps{i}")
            for i in range(3)]
    for db in range(3):
        cbs = [cb for cb in range(3) if abs(cb - db) <= 1]
        for idx, cb in enumerate(cbs):
            nc.tensor.matmul(
                m_ps[db][:], lhsT=attn_t[cb][:, db * 128:(db + 1) * 128],
                rhs=wp_sb[:, cb * 384:(cb + 1) * 384],
                start=(idx == 0), stop=(idx == len(cbs) - 1))
    m_sb = cst.tile([128, 3 * 384], dt.bfloat16, tag="msb")
    for db in range(3):
        nc.scalar.copy(m_sb[:, db * 384:(db + 1) * 384], m_ps[db][:])
    mp.release()

    # ---------------- out = M^T @ V (bf16 out) + store ----------------
    otp = tc.alloc_tile_pool(name="otp", bufs=2)
    for ob in range(3):
        for t in range(16):
            ps = mmp.tile([128, 512], dt.float32, tag="mm")
            hh, r4 = t // 8, (t % 8) * 4
            for db in range(3):
                vv = dwv[db][:, hh * DWH:(hh + 1) * DWH].rearrange(
                    "p (r c) -> p r c", c=WP)
                nc.tensor.matmul(
                    ps[:],
                    lhsT=m_sb[:, db * 384 + ob * 128: db * 384 + ob * 128 + 128],
                    rhs=vv[:, r4:r4 + 4, 0:128],
                    start=(db == 0), stop=(db == 2))
            ot = otp.tile([128, 512], dt.bfloat16, tag="ot")
            ecopy(OUT_ROT, ot[:], ps[:])
            nc.sync.dma_start(
                out=outd.ap()[ob, :, 512 * t:512 * (t + 1)], in_=ot[:])

    for p in (otp, smp, drp, mmp, accp, vpp, qpp, xp16, xp8, wk, dwp, cst):
        p.release()


def build_nc():
    if "nc" in _CACHE:
        return _CACHE["nc"]
    from concourse import bacc, tile
    import concourse.mybir as mybir
    dt = mybir.dt
    nc = bacc.Bacc("TRN2", target_bir_lowering=False, debug=False, num_devices=8)
    xd8 = nc.dram_tensor("x8", [3, 128, 66 * WP], dt.float8e4,
                         kind="ExternalInput")
    xd16 = nc.dram_tensor("x16", [3, 128, 66 * WP], dt.bfloat16,
                          kind="ExternalInput")
    wqd = nc.dram_tensor("wq8", [128, 6 * 512], dt.float8e4,
                         kind="ExternalInput")
    wvd = nc.dram_tensor("wv", [128, 3 * 384], dt.bfloat16,
                         kind="ExternalInput")
    dtd = nc.dram_tensor("dt8", [128, 6 * 1280], dt.float8e4,
                         kind="ExternalInput")
    dtvd = nc.dram_tensor("dtv", [128, 3 * len(V_PE) * 128], dt.bfloat16,
                          kind="ExternalInput")
    dsd = nc.dram_tensor("dwsc", [128, 27], dt.float32,
                         kind="ExternalInput")
    wpd = nc.dram_tensor("wp", [128, 3 * 384], dt.bfloat16,
                         kind="ExternalInput")
    idd = nc.dram_tensor("identb", [128, 128], dt.bfloat16,
                         kind="ExternalInput")
    idud = nc.dram_tensor("idu", [128, 128], dt.float8e4,
                          kind="ExternalInput")
    mkd = nc.dram_tensor("maskt", [128, 3 * 384], dt.bfloat16,
                         kind="ExternalInput")
    tpd = nc.dram_tensor("tempc", [128, 4], dt.float32, kind="ExternalInput")
    outd = nc.dram_tensor("out", [3, 128, NT], dt.bfloat16,
                          kind="ExternalOutput")
    with tile.TileContext(nc) as tc:
        _build_body(nc, tc, (xd8, xd16, wqd, wvd, dtd, dtvd, dsd, wpd, idd,
                             idud, mkd, tpd, outd))
    nc.compile()
    _CACHE["nc"] = nc
    return nc


def make_in_maps(x, qkv_w, dw_w, proj_w, temperature):
    x = np.asarray(x, np.float32)
    qkv_w = np.asarray(qkv_w, np.float32)
    dw_w = np.asarray(dw_w, np.float32)
    proj_w = np.asarray(proj_w, np.float32)
    temperature = np.asarray(temperature, np.float32).reshape(-1)

    xp = np.zeros((B, C, 130, 130), np.float32)
    xp[:, :, 1:129, 1:129] = x

    rng = np.arange(128)
    # qk conv weights, fp8 DoubleRow; unit2 = [zeros | W-block-2]
    wq8 = np.zeros((128, 6 * 512), np.float32)
    for ob in range(6):
        blk = qkv_w[ob * 128:(ob + 1) * 128] * SCW  # [128m, 384c]
        for i in range(2):
            wq8[:, ob * 512 + i * 128: ob * 512 + (i + 1) * 128] = \
                blk[:, i * 128:(i + 1) * 128].T
        wq8[:, ob * 512 + 384: ob * 512 + 512] = blk[:, 256:384].T
    # v conv weights bf16 (classic layout)
    wv16 = np.zeros((128, 3 * 384), np.float32)
    for vb in range(3):
        for cb in range(3):
            blk = qkv_w[768 + vb * 128: 768 + (vb + 1) * 128,
                        cb * 128:(cb + 1) * 128]
            wv16[:, vb * 384 + cb * 128: vb * 384 + (cb + 1) * 128] = blk.T
    # qk dw tap diagonals, fp8 DoubleRow; unit4 = [zeros | single tap]
    dt8 = np.zeros((128, 6 * 1280), np.float32)
    for ob in range(6):
        for u, (t1, t2) in enumerate(QK_PAIRS):
            c0 = ob * 1280 + u * 256
            dt8[rng, c0 + rng] = dw_w[ob * 128 + rng, 0, t1[0], t1[1]] * SCD
            dt8[rng, c0 + 128 + rng] = dw_w[ob * 128 + rng, 0, t2[0], t2[1]] * SCD
        c0 = ob * 1280 + 1024 + 128
        dt8[rng, c0 + rng] = dw_w[ob * 128 + rng, 0, QK_SINGLE[0],
                                  QK_SINGLE[1]] * SCD
    # v dw PE tap diagonals bf16
    dtv16 = np.zeros((128, 3 * len(V_PE) * 128), np.float32)
    for vb in range(3):
        for t, (dy, dx) in enumerate(V_PE):
            c0 = (vb * len(V_PE) + t) * 128
            dtv16[rng, c0 + rng] = dw_w[768 + vb * 128 + rng, 0, dy, dx]
    # v dw per-channel scales (DVE/pool taps)
    dwsc = np.zeros((128, 27), np.float32)
    for vb in range(3):
        for k9 in range(9):
            dwsc[:, vb * 9 + k9] = dw_w[768 + vb * 128:768 + (vb + 1) * 128,
                                        0, k9 // 3, k9 % 3]
    wpm = np.zeros((128, 3 * 384), np.float32)
    for cb in range(3):
        wpm[:, cb * 384:(cb + 1) * 384] = proj_w[:, cb * 128:(cb + 1) * 128].T
    ident = np.eye(128, dtype=np.float32)
    idu8 = np.eye(128, dtype=np.float32).astype(F8)
    mask = np.full((128, 3 * 384), -1e30, np.float32)
    for j in range(3):
        for p in range(128):
            hgrp = (128 * j + p) // CHD
            mask[p, 384 * j + CHD * hgrp: 384 * j + CHD * (hgrp + 1)] = 0.0
    tempc = np.zeros((128, 4), np.float32)
    for j in range(3):
        for p in range(128):
            tempc[p, j] = temperature[(128 * j + p) // CHD] / 256.0

    shared = {
        "wq8": wq8.astype(F8), "wv": wv16.astype(BF16),
        "dt8": dt8.astype(F8), "dtv": dtv16.astype(BF16),
        "dwsc": dwsc,
        "wp": wpm.astype(BF16), "identb": ident.astype(BF16), "idu": idu8,
        "maskt": mask.astype(BF16), "tempc": tempc,
    }
    in_maps = []
    for core in range(8):
        b, s = core // 2, core % 2
        xs = xp[b, :, 64 * s: 64 * s + 66, :]
        xs = np.ascontiguousarray(xs.reshape(3, 128, 66 * WP))
        m = dict(shared)
        m["x8"] = xs.astype(F8)
        m["x16"] = xs.astype(BF16)
        in_maps.append(m)
    return in_maps


def assemble(results):
    full = np.zeros((B, C, 128, 128), np.float32)
    for core in range(8):
        b, s = core // 2, core % 2
        o = np.asarray(results[core]["out"], np.float32).reshape(C, 64, 128)
        full[b, :, 64 * s: 64 * s + 64, :] = o
    return full


def kernel(x, qkv_w, dw_w, proj_w, temperature):
    from concourse.bass_utils import run_bass_kernel_spmd
    nc = build_nc()
    in_maps = make_in_maps(x, qkv_w, dw_w, proj_w, temperature)
    res = run_bass_kernel_spmd(nc, in_maps, core_ids=list(range(8)))
    return assemble(res.results)
